# revision 1
# baseline (speedup 1.0000x reference)
"""Multi-head attention TRN2 kernel (8 NeuronCores, SPMD).

Problem: B=2, N=2048, D=1024, H=16 heads of dim 64, fp32, per-(b,h)
key-length masking (valid_len, length 32).

Sharding: batch*heads across 8 cores — core c handles batch b=c//4 and 4
heads ("slots").  Per core:
  Q^T/K^T = Wslice^T @ x^T   (heads on partitions, positions on free)
  V       = x^T-tiles as lhsT, Wv as rhs  (positions on partitions)
  S^T     = K^T.T @ Q^T  per head (row-packed K=64 pairs)
  P^T     = exp(S^T / 8) (ScalarE, fused scale)
  heads^T = [V|1].T @ P^T  accumulated over key tiles (ones column gives
            the softmax denominator as row 64; masking is folded into V
            by zeroing rows >= valid_len via a per-partition mask scale)
  normalize via DVE reciprocal + one Newton step, PE K=1 broadcast
  out_partial = heads^T.T @ Wo_slice  -> (2048, 1024) per core
Host sums the 4 per-core partials of each batch element (the unshard for
the row-sharded Wo) and gathers.

All matmuls run as float32r (TF32-like, ~1e-4 rel err, full PE rate).
The per-head key loop is specialized at build time to
ceil(max_vl_in_slot/128) tiles; exact masking comes from the mask scale.
"""
import sys
import numpy as np
from contextlib import ExitStack

sys.path.insert(0, "/opt/trn_rl_repo")

import concourse.bass as bass  # noqa: E402
from concourse import bacc, mybir  # noqa: E402
import concourse.tile as tile  # noqa: E402
from concourse.bass_utils import run_bass_kernel_spmd  # noqa: E402

F32 = mybir.dt.float32
F32R = mybir.dt.float32r
BF16 = mybir.dt.bfloat16
AF = mybir.ActivationFunctionType

B, N, D, H = 2, 2048, 1024, 16
DH = 64
HPC = 4          # heads (slots) per core
NCORES = 8
QC = 512         # q chunk (matmul free dim)
NQT = N // 128   # 16 q tiles
NKT = N // 128   # 16 k tiles
NDC = D // 128   # 8 contraction chunks

LAST_RESULTS = None  # BassKernelResults of the most recent run (for tooling)


def _build_program(trips):
    """trips: 4 ints — k-tile count per slot (uniform across cores)."""
    nc = bacc.Bacc("TRN2", target_bir_lowering=False, debug=False,
                   num_devices=NCORES)

    xTq = nc.dram_tensor("xTq", [D, N], F32R, kind="ExternalInput")
    xTk = nc.dram_tensor("xTk", [D, N], F32R, kind="ExternalInput")
    xTv = nc.dram_tensor("xTv", [D, N], F32R, kind="ExternalInput")
    wq = nc.dram_tensor("wq", [128, NDC * 256], F32R, kind="ExternalInput")
    wk = nc.dram_tensor("wk", [128, NDC * 256], F32R, kind="ExternalInput")
    wv = nc.dram_tensor("wv", [128, NDC * 256], F32R, kind="ExternalInput")
    wo = nc.dram_tensor("wo", [256, D], F32R, kind="ExternalInput")
    vmask = nc.dram_tensor("vmask", [128, HPC * NKT], F32, kind="ExternalInput")
    out = nc.dram_tensor("out", [N, D], F32, kind="ExternalOutput")

    with tile.TileContext(nc) as tc:
        with ExitStack() as ctx:
            wpool = ctx.enter_context(tc.tile_pool(name="wpool", bufs=1))
            xpool = ctx.enter_context(tc.tile_pool(name="xpool", bufs=3))
            qkpool = ctx.enter_context(tc.tile_pool(name="qkpool", bufs=1))
            v1pool = ctx.enter_context(tc.tile_pool(name="v1pool", bufs=1))
            ptpool = ctx.enter_context(tc.tile_pool(name="ptpool", bufs=4))
            nrmpool = ctx.enter_context(tc.tile_pool(name="nrmpool", bufs=2))
            pbpool = ctx.enter_context(tc.tile_pool(name="pbpool", bufs=1))
            opool = ctx.enter_context(tc.tile_pool(name="opool", bufs=3))

            t_wq = wpool.tile([128, NDC * 256], F32R, tag="wq")
            t_wk = wpool.tile([128, NDC * 256], F32R, tag="wk")
            t_wv = wpool.tile([128, NDC * 256], F32R, tag="wv")
            nc.sync.dma_start(t_wq[:], wq[:])
            nc.sync.dma_start(t_wk[:], wk[:])
            nc.sync.dma_start(t_wv[:], wv[:])
            t_wo = [wpool.tile([128, D], F32R, tag=f"wo{p}", name=f"t_wo{p}") for p in range(2)]
            nc.sync.dma_start(t_wo[0][:], wo[0:128, :])
            nc.sync.dma_start(t_wo[1][:], wo[128:256, :])
            t_vm = wpool.tile([128, HPC * NKT], F32, tag="vm")
            nc.sync.dma_start(t_vm[:], vmask[:])

            # Q^T / K^T: [128 dims (2 slots), N] per slot-pair
            t_qT = [qkpool.tile([128, N], F32R, tag=f"qT{p}", name=f"t_qT{p}") for p in range(2)]
            t_kT = [qkpool.tile([128, N], F32R, tag=f"kT{p}", name=f"t_kT{p}") for p in range(2)]
            # V1 per slot: NKT tiles of [128, 65] ([V | ones-masked])
            t_v1 = [v1pool.tile([128, 65 * trips[j]], F32R, tag=f"v1_{j}", name=f"t_v1_{j}")
                    for j in range(HPC)]
            # normalized heads^T per slot pair: [128 dims, N]
            t_pb = [pbpool.tile([128, N], F32R, tag=f"pb{p}", name=f"t_pb{p}") for p in range(2)]

            # ---- phase 1: projections ----
            with tc.tile_pool(name="pp", bufs=8, space="PSUM") as pp:
                for xin, wsb, dsts in ((xTq, t_wq, t_qT), (xTk, t_wk, t_kT)):
                    accs = [pp.tile([128, QC], F32, tag="acc", name=f"acc_{i}")
                            for i in range(2 * (N // QC))]
                    for c in range(NDC):
                        xt = xpool.tile([128, N], F32R, tag="xt")
                        nc.sync.dma_start(xt[:], xin[c * 128:(c + 1) * 128, :])
                        for m in range(2):
                            for q in range(N // QC):
                                nc.tensor.matmul(
                                    accs[m * (N // QC) + q][:],
                                    wsb[:, c * 256 + m * 128: c * 256 + (m + 1) * 128],
                                    xt[:, q * QC:(q + 1) * QC],
                                    start=(c == 0), stop=(c == NDC - 1))
                    for m in range(2):
                        for q in range(N // QC):
                            i = m * (N // QC) + q
                            dst = dsts[m][:, q * QC:(q + 1) * QC]
                            if i % 2 == 0:
                                nc.scalar.activation(dst, accs[i][:], AF.Copy)
                            else:
                                with nc.allow_low_precision(reason="f32r 4B"):
                                    nc.vector.tensor_copy(dst, accs[i][:])
                # V projection: two half-column passes of 8 k-tiles
                for g in range(2):
                    accs = [pp.tile([128, 256], F32, tag="acc", name=f"accv_{i}") for i in range(8)]
                    for c in range(NDC):
                        xt = xpool.tile([128, 1024], F32R, tag="xtv")
                        nc.sync.dma_start(
                            xt[:], xTv[c * 128:(c + 1) * 128,
                                       g * 1024:(g + 1) * 1024])
                        for kt8 in range(8):
                            nc.tensor.matmul(
                                accs[kt8][:],
                                xt[:, kt8 * 128:(kt8 + 1) * 128],
                                wsb_v_rhs(t_wv, c),
                                start=(c == 0), stop=(c == NDC - 1))
                    for kt8 in range(8):
                        t = g * 8 + kt8
                        for j in range(HPC):
                            if t >= trips[j]:
                                continue
                            mask_col = t_vm[:, j * NKT + t: j * NKT + t + 1]
                            # V columns scaled by mask (zero rows >= vl)
                            nc.scalar.activation(
                                t_v1[j][:, t * 65: t * 65 + 64],
                                accs[kt8][:, j * 64:(j + 1) * 64],
                                AF.Copy, scale=mask_col)
                            # ones column = mask itself
                            nc.vector.tensor_copy(
                                t_v1[j][:, t * 65 + 64: t * 65 + 65], mask_col)

            # ---- phase 2+3: attention with fused output projection ----
            with tc.tile_pool(name="ap", bufs=1, space="PSUM") as ap:
                for q in range(N // QC):
                    qs = slice(q * QC, (q + 1) * QC)
                    for p in range(2):
                        ja, jb = 2 * p, 2 * p + 1
                        acc_a = ap.tile([65, QC], F32, tag="acc2", bufs=4,
                                        name="acc_a")
                        acc_b = ap.tile([65, QC], F32, tag="acc2", bufs=4,
                                        name="acc_b")
                        for t in range(trips[ja]):
                            ks = slice(t * 128, (t + 1) * 128)
                            sT_a = ap.tile([128, QC], F32, tag="sT", bufs=2)
                            nc.tensor.matmul(sT_a[:], t_kT[p][0:64, ks],
                                             t_qT[p][0:64, qs],
                                             start=True, stop=True)
                            pT_a = ptpool.tile([128, QC], F32R, tag="pT")
                            nc.scalar.activation(pT_a[:], sT_a[:], AF.Exp,
                                                 scale=0.125)
                            nc.tensor.matmul(
                                acc_a[:], t_v1[ja][:, t * 65:(t + 1) * 65],
                                pT_a[:], start=(t == 0),
                                stop=(t == trips[ja] - 1))
                            if t < trips[jb]:
                                sT_b = ap.tile([128, QC], F32, tag="sT", bufs=2)
                                nc.tensor.matmul(sT_b[:], t_kT[p][64:128, ks],
                                                 t_qT[p][64:128, qs],
                                                 start=True, stop=True)
                                pT_b = ptpool.tile([128, QC], F32R, tag="pT")
                                nc.scalar.activation(pT_b[:], sT_b[:], AF.Exp,
                                                     scale=0.125)
                                nc.tensor.matmul(
                                    acc_b[:], t_v1[jb][:, t * 65:(t + 1) * 65],
                                    pT_b[:], start=(t == 0),
                                    stop=(t == trips[jb] - 1))
                        # normalize both slots of the pair (bcast on GpSimd)
                        for e, acc in ((0, acc_a), (1, acc_b)):
                            r0 = nrmpool.tile([1, QC], F32, tag="r0")
                            nc.vector.reciprocal(r0[:], acc[64:65, :])
                            # Newton: r1 = r0 * (2 - d*r0)
                            t1 = nrmpool.tile([1, QC], F32, tag="t1")
                            nc.vector.tensor_mul(t1[:], acc[64:65, :], r0[:])
                            t2 = nrmpool.tile([1, QC], F32, tag="t2")
                            nc.vector.tensor_scalar(
                                t2[:], t1[:], -1.0, 2.0,
                                mybir.AluOpType.mult, mybir.AluOpType.add)
                            r1 = nrmpool.tile([1, QC], F32, tag="r1")
                            nc.vector.tensor_mul(r1[:], r0[:], t2[:])
                            bc_sb = nrmpool.tile([64, QC], F32, tag="bc_sb")
                            nc.gpsimd.partition_broadcast(bc_sb[:], r1[:])
                            with nc.allow_low_precision(reason="f32r 4B"):
                                nc.vector.tensor_mul(
                                    t_pb[p][e * 64:(e + 1) * 64, qs],
                                    acc[0:64, :], bc_sb[:])
                    # output projection for the 4 q-tiles of this chunk
                    for qt in range(q * (QC // 128), (q + 1) * (QC // 128)):
                        ts = slice(qt * 128, (qt + 1) * 128)
                        stage = opool.tile([128, D], F32, tag="ostage")
                        for ch in range(2):
                            o_ps = ap.tile([128, 512], F32, tag="o", bufs=2)
                            for p in range(2):
                                nc.tensor.matmul(
                                    o_ps[:], t_pb[p][:, ts],
                                    t_wo[p][:, ch * 512:(ch + 1) * 512],
                                    start=(p == 0), stop=(p == 1))
                            nc.vector.tensor_copy(
                                stage[:, ch * 512:(ch + 1) * 512], o_ps[:])
                        nc.sync.dma_start(out[ts, :], stage[:])

    nc.finalize()
    return nc


def wsb_v_rhs(t_wv, c):
    return t_wv[:, c * 256:(c + 1) * 256]


def kernel(queries, keys, values, valid_len, Wq, Wk, Wv, Wo):
    global LAST_RESULTS
    queries = np.asarray(queries, dtype=np.float32)
    keys = np.asarray(keys, dtype=np.float32)
    values = np.asarray(values, dtype=np.float32)
    Wq = np.asarray(Wq, dtype=np.float32)
    Wk = np.asarray(Wk, dtype=np.float32)
    Wv = np.asarray(Wv, dtype=np.float32)
    Wo = np.asarray(Wo, dtype=np.float32)
    vl = np.asarray(valid_len).astype(np.int64).reshape(B * H)

    # rank-aligned slot assignment: per batch, heads sorted by vl desc;
    # slot j of the 4 cores of that batch takes ranks 4j..4j+3
    order = {}
    for b in range(B):
        idx = (np.argsort(-vl[b * H:(b + 1) * H], kind="stable") + b * H)
        for cg in range(4):
            order[b * 4 + cg] = [int(idx[4 * j + cg]) for j in range(HPC)]
    trips = []
    for j in range(HPC):
        m = max(int(-(-vl[order[c][j]] // 128)) for c in range(NCORES))
        trips.append(max(1, min(NKT, m)))

    nc = _build_program(tuple(trips))

    in_maps = []
    for c in range(NCORES):
        b = c // 4
        heads = order[c]
        cols = np.concatenate(
            [np.arange((h - b * H) * DH, (h - b * H + 1) * DH) for h in heads])

        def wlayout(w):
            return np.ascontiguousarray(
                w[:, cols].reshape(NDC, 128, 256).transpose(1, 0, 2)
                .reshape(128, NDC * 256))

        vm = np.zeros((128, HPC * NKT), np.float32)
        for j, h in enumerate(heads):
            keep = (np.arange(N) < vl[h]).astype(np.float32)
            vm[:, j * NKT:(j + 1) * NKT] = keep.reshape(NKT, 128).T

        in_maps.append({
            "xTq": np.ascontiguousarray(queries[b].T),
            "xTk": np.ascontiguousarray(keys[b].T),
            "xTv": np.ascontiguousarray(values[b].T),
            "wq": wlayout(Wq),
            "wk": wlayout(Wk),
            "wv": wlayout(Wv),
            "wo": np.ascontiguousarray(Wo[cols, :]),
            "vmask": vm,
        })

    LAST_RESULTS = run_bass_kernel_spmd(nc, in_maps, list(range(NCORES)))
    res = LAST_RESULTS.results

    out = np.zeros((B, N, D), np.float64)
    for c in range(NCORES):
        out[c // 4] += res[c]["out"].astype(np.float64)
    return out.astype(np.float32)



# revision 3
# speedup vs baseline: 1.0257x; 1.0257x over previous
"""Multi-head attention TRN2 kernel (8 NeuronCores, SPMD).

Problem: B=2, N=2048, D=1024, H=16 heads of dim 64, fp32, per-(b,h)
key-length masking (valid_len, length 32).

Sharding: batch*heads across 8 cores - core c handles batch b=c//4 and 4
heads ("slots", rank-aligned by valid_len so the SPMD trip counts stay
balanced).  Per core:

  phase P (projections, bf16 inputs to halve HBM traffic):
    K^T/Q^T = Wslice^T @ x^T   (head dims on partitions, positions free)
    V       = x^T-tiles as lhsT, Wv as rhs  (positions on partitions),
              copied into V1 = [V_j | 1] blocks per (key-tile, slot)
  phase A (attention, f32r):
    S^T   = K^T.T @ Q^T per (slot, key-tile), batched in PSUM pairs
    P^T   = exp(S^T/8 + bias) on ScalarE - the valid_len mask is a
            per-partition bias column (0 / -30000), so no V masking ops;
            key-tile pairs that are uniformly valid/invalid across all
            cores share one exp instruction ([128,1024] batch)
    acc   = V1.T @ P^T accumulated over key tiles (ones column gives the
            softmax denominator as row 64)
    normalize via DVE reciprocal + Newton, GpSimd partition broadcast
    out_partial = heads^T.T @ Wo_slice -> (2048, 1024) per core
Host sums the 4 per-core partials of each batch element (the unshard for
the row-sharded Wo) and gathers.

Matmuls: projections run bf16*bf16 (f32 accumulate); attention runs
float32r (full PE rate at free>=256).
"""
import sys
import numpy as np
from contextlib import ExitStack

sys.path.insert(0, "/opt/trn_rl_repo")

import concourse.bass as bass  # noqa: E402
from concourse import bacc, mybir  # noqa: E402
import concourse.tile as tile  # noqa: E402
from concourse.bass_utils import run_bass_kernel_spmd  # noqa: E402

F32 = mybir.dt.float32
F32R = mybir.dt.float32r
BF16 = mybir.dt.bfloat16
AF = mybir.ActivationFunctionType
NPBF16 = mybir.dt.np(BF16)

B, N, D, H = 2, 2048, 1024, 16
DH = 64
HPC = 4          # heads (slots) per core
NCORES = 8
QC = 512         # q chunk (matmul free dim)
NKT = N // 128   # 16 k tiles
NDC = D // 128   # 8 contraction chunks
MASK_BIAS = -30000.0

LAST_RESULTS = None  # BassKernelResults of the most recent run (for tooling)


def _build_program(trips, plans):
    """trips: 4 ints (k-tile count per slot); plans: per slot, list of
    (t0, ntiles) exp-batch groups covering range(trips[j])."""
    nc = bacc.Bacc("TRN2", target_bir_lowering=False, debug=False,
                   num_devices=NCORES)

    xTq = nc.dram_tensor("xTq", [D, N], BF16, kind="ExternalInput")
    xTk = nc.dram_tensor("xTk", [D, N], BF16, kind="ExternalInput")
    xTv = nc.dram_tensor("xTv", [D, N], BF16, kind="ExternalInput")
    wq = nc.dram_tensor("wq", [128, NDC * 256], BF16, kind="ExternalInput")
    wk = nc.dram_tensor("wk", [128, NDC * 256], BF16, kind="ExternalInput")
    wv = nc.dram_tensor("wv", [128, NDC * 256], BF16, kind="ExternalInput")
    wo = nc.dram_tensor("wo", [256, D], F32R, kind="ExternalInput")
    vmask = nc.dram_tensor("vmask", [128, HPC * NKT], F32, kind="ExternalInput")
    out = nc.dram_tensor("out", [N, D], F32, kind="ExternalOutput")

    with tile.TileContext(nc) as tc:
        with ExitStack() as ctx:
            wpool = ctx.enter_context(tc.tile_pool(name="wpool", bufs=1))
            xpool = ctx.enter_context(tc.tile_pool(name="xpool", bufs=3))
            qkpool = ctx.enter_context(tc.tile_pool(name="qkpool", bufs=1))
            v1pool = ctx.enter_context(tc.tile_pool(name="v1pool", bufs=1))
            ptpool = ctx.enter_context(tc.tile_pool(name="ptpool", bufs=4))
            nrmpool = ctx.enter_context(tc.tile_pool(name="nrmpool", bufs=2))
            pbpool = ctx.enter_context(tc.tile_pool(name="pbpool", bufs=1))
            opool = ctx.enter_context(tc.tile_pool(name="opool", bufs=3))

            t_wk = wpool.tile([128, NDC * 256], BF16, tag="wk")
            nc.sync.dma_start(t_wk[:], wk[:])
            t_wq = wpool.tile([128, NDC * 256], BF16, tag="wq")
            nc.sync.dma_start(t_wq[:], wq[:])
            t_wv = wpool.tile([128, NDC * 256], BF16, tag="wv")
            nc.sync.dma_start(t_wv[:], wv[:])
            t_wo = [wpool.tile([128, D], F32R, tag=f"wo{p}", name=f"t_wo{p}")
                    for p in range(2)]
            nc.sync.dma_start(t_wo[0][:], wo[0:128, :])
            nc.sync.dma_start(t_wo[1][:], wo[128:256, :])
            t_vm = wpool.tile([128, HPC * NKT], F32, tag="vm")
            nc.sync.dma_start(t_vm[:], vmask[:])

            # K^T/Q^T: [128 dims (2 slots), N] per slot-pair
            t_kT = [qkpool.tile([128, N], F32R, tag=f"kT{p}", name=f"t_kT{p}")
                    for p in range(2)]
            t_qT = [qkpool.tile([128, N], F32R, tag=f"qT{p}", name=f"t_qT{p}")
                    for p in range(2)]
            # V1: per key-tile t, 4 blocks of [V_j (64 cols) | ones (1 col)]
            t_v1 = v1pool.tile([128, NKT * HPC * 65], F32R, tag="v1")
            # normalized heads^T per slot pair: [128 dims, N]
            t_pb = [pbpool.tile([128, N], F32R, tag=f"pb{p}", name=f"t_pb{p}")
                    for p in range(2)]

            # ones columns of V1, one strided memset (f32 view: f32r is not
            # a valid memset value type)
            ones_ap = t_v1[:].bitcast(F32).rearrange(
                "p (b c) -> p b c", c=65)[:, :, 64:65]
            nc.vector.memset(ones_ap, 1.0)

            # ---- phase P: projections (K, V, Q) ----
            with tc.tile_pool(name="pp", bufs=8, space="PSUM") as pp:
                for xin, wsb, dsts in ((xTk, t_wk, t_kT), (xTq, t_wq, t_qT)):
                    accs = [pp.tile([128, QC], F32, tag="acc", name=f"acc_{i}")
                            for i in range(8)]
                    for c in range(NDC):
                        xt = xpool.tile([128, N], BF16, tag="xt")
                        nc.sync.dma_start(xt[:], xin[c * 128:(c + 1) * 128, :])
                        for m in range(2):
                            for qq in range(4):
                                nc.tensor.matmul(
                                    accs[m * 4 + qq][:],
                                    wsb[:, c * 256 + m * 128:
                                        c * 256 + (m + 1) * 128],
                                    xt[:, qq * QC:(qq + 1) * QC],
                                    start=(c == 0), stop=(c == NDC - 1))
                    for i in range(8):
                        dst = dsts[i // 4][:, (i % 4) * QC:(i % 4 + 1) * QC]
                        with nc.allow_low_precision(reason="f32r 4B"):
                            nc.vector.tensor_copy(dst, accs[i][:])
                # V projection: two half-column passes of 8 k-tiles
                for g in range(2):
                    accs = [pp.tile([128, 256], F32, tag="acc",
                                    name=f"accv_{i}") for i in range(8)]
                    for c in range(NDC):
                        xt = xpool.tile([128, 1024], BF16, tag="xtv")
                        nc.sync.dma_start(
                            xt[:], xTv[c * 128:(c + 1) * 128,
                                       g * 1024:(g + 1) * 1024])
                        for kt8 in range(8):
                            nc.tensor.matmul(
                                accs[kt8][:],
                                xt[:, kt8 * 128:(kt8 + 1) * 128],
                                t_wv[:, c * 256:(c + 1) * 256],
                                start=(c == 0), stop=(c == NDC - 1))
                    for kt8 in range(8):
                        t = g * 8 + kt8
                        # [128, 4, 64] strided copy: slot j -> V1 block
                        src = accs[kt8][:].rearrange("p (j c) -> p j c", c=64)
                        dst = t_v1[:, t * 260:(t + 1) * 260].rearrange(
                            "p (j c) -> p j c", c=65)[:, :, 0:64]
                        with nc.allow_low_precision(reason="f32r 4B"):
                            nc.vector.tensor_copy(dst, src)

            # ---- phase A: attention with fused output projection ----
            with tc.tile_pool(name="ap", bufs=1, space="PSUM") as ap:
                for q in range(N // QC):
                    qs = slice(q * QC, (q + 1) * QC)
                    for j in range(HPC):
                        p, half = j // 2, j % 2
                        rows = slice(half * 64, (half + 1) * 64)
                        acc = ap.tile([65, QC], F32, tag="acc2", bufs=4,
                                      name=f"acc_{j}")
                        plan = plans[j]
                        for gi, (t0, nt) in enumerate(plan):
                            sT = ap.tile([128, nt * QC], F32, tag="sT", bufs=2)
                            for i in range(nt):
                                t = t0 + i
                                nc.tensor.matmul(
                                    sT[:, i * QC:(i + 1) * QC],
                                    t_kT[p][rows, t * 128:(t + 1) * 128],
                                    t_qT[p][rows, qs],
                                    start=True, stop=True)
                            pT = ptpool.tile([128, nt * QC], F32R, tag="pT")
                            nc.scalar.activation(
                                pT[:], sT[:], AF.Exp, scale=0.125,
                                bias=t_vm[:, j * NKT + t0: j * NKT + t0 + 1])
                            for i in range(nt):
                                t = t0 + i
                                base = (t * HPC + j) * 65
                                nc.tensor.matmul(
                                    acc[:], t_v1[:, base: base + 65],
                                    pT[:, i * QC:(i + 1) * QC],
                                    start=(gi == 0 and i == 0),
                                    stop=(gi == len(plan) - 1 and i == nt - 1))
                        # normalize (recip + one Newton step, bcast on GpSimd)
                        r0 = nrmpool.tile([1, QC], F32, tag="r0")
                        nc.vector.reciprocal(r0[:], acc[64:65, :])
                        t1 = nrmpool.tile([1, QC], F32, tag="t1")
                        nc.vector.tensor_mul(t1[:], acc[64:65, :], r0[:])
                        t2 = nrmpool.tile([1, QC], F32, tag="t2")
                        nc.vector.tensor_scalar(
                            t2[:], t1[:], -1.0, 2.0,
                            mybir.AluOpType.mult, mybir.AluOpType.add)
                        r1 = nrmpool.tile([1, QC], F32, tag="r1")
                        nc.vector.tensor_mul(r1[:], r0[:], t2[:])
                        bc_sb = nrmpool.tile([64, QC], F32, tag="bc_sb")
                        nc.gpsimd.partition_broadcast(bc_sb[:], r1[:])
                        with nc.allow_low_precision(reason="f32r 4B"):
                            nc.vector.tensor_mul(
                                t_pb[p][rows, qs], acc[0:64, :], bc_sb[:])
                    # output projection for the 4 q-tiles of this chunk
                    for qt in range(q * (QC // 128), (q + 1) * (QC // 128)):
                        ts = slice(qt * 128, (qt + 1) * 128)
                        stage = opool.tile([128, D], F32, tag="ostage")
                        for ch in range(2):
                            o_ps = ap.tile([128, 512], F32, tag="sT", bufs=2)
                            for p2 in range(2):
                                nc.tensor.matmul(
                                    o_ps[:], t_pb[p2][:, ts],
                                    t_wo[p2][:, ch * 512:(ch + 1) * 512],
                                    start=(p2 == 0), stop=(p2 == 1))
                            nc.any.tensor_copy(
                                stage[:, ch * 512:(ch + 1) * 512], o_ps[:])
                        nc.sync.dma_start(out[ts, :], stage[:])

    nc.finalize()
    return nc


def _make_plans(trips, vls_by_slot):
    """Greedy pair batching: (t, t+1) share one exp iff every core's vl is
    outside the open interval (128*t, 128*(t+2)) - then one bias column
    describes both tiles on every core."""
    plans = []
    for j in range(HPC):
        plan, t = [], 0
        while t < trips[j]:
            if t + 1 < trips[j] and all(
                    v <= 128 * t or v >= 128 * (t + 2)
                    for v in vls_by_slot[j]):
                plan.append((t, 2))
                t += 2
            else:
                plan.append((t, 1))
                t += 1
        plans.append(plan)
    return plans


def kernel(queries, keys, values, valid_len, Wq, Wk, Wv, Wo):
    global LAST_RESULTS
    queries = np.asarray(queries, dtype=np.float32)
    keys = np.asarray(keys, dtype=np.float32)
    values = np.asarray(values, dtype=np.float32)
    Wq = np.asarray(Wq, dtype=np.float32)
    Wk = np.asarray(Wk, dtype=np.float32)
    Wv = np.asarray(Wv, dtype=np.float32)
    Wo = np.asarray(Wo, dtype=np.float32)
    vl = np.asarray(valid_len).astype(np.int64).reshape(B * H)

    # rank-aligned slot assignment: per batch, heads sorted by vl desc;
    # slot j of the 4 cores of that batch takes ranks 4j..4j+3
    order = {}
    for b in range(B):
        idx = (np.argsort(-vl[b * H:(b + 1) * H], kind="stable") + b * H)
        for cg in range(4):
            order[b * 4 + cg] = [int(idx[4 * j + cg]) for j in range(HPC)]
    trips, vls_by_slot = [], []
    for j in range(HPC):
        vs = [int(vl[order[c][j]]) for c in range(NCORES)]
        vls_by_slot.append(vs)
        m = max(-(-v // 128) for v in vs)
        trips.append(max(1, min(NKT, m)))
    plans = _make_plans(trips, vls_by_slot)

    nc = _build_program(tuple(trips), plans)

    in_maps = []
    for c in range(NCORES):
        b = c // 4
        heads = order[c]
        cols = np.concatenate(
            [np.arange((h - b * H) * DH, (h - b * H + 1) * DH) for h in heads])

        def wlayout(w):
            return np.ascontiguousarray(
                w[:, cols].reshape(NDC, 128, 256).transpose(1, 0, 2)
                .reshape(128, NDC * 256).astype(NPBF16))

        vm = np.zeros((128, HPC * NKT), np.float32)
        for j, h in enumerate(heads):
            bias = np.where(np.arange(N) < vl[h], 0.0, MASK_BIAS)
            vm[:, j * NKT:(j + 1) * NKT] = bias.reshape(NKT, 128).T

        in_maps.append({
            "xTq": np.ascontiguousarray(queries[b].T.astype(NPBF16)),
            "xTk": np.ascontiguousarray(keys[b].T.astype(NPBF16)),
            "xTv": np.ascontiguousarray(values[b].T.astype(NPBF16)),
            "wq": wlayout(Wq),
            "wk": wlayout(Wk),
            "wv": wlayout(Wv),
            "wo": np.ascontiguousarray(Wo[cols, :]),
            "vmask": vm,
        })

    LAST_RESULTS = run_bass_kernel_spmd(nc, in_maps, list(range(NCORES)))
    res = LAST_RESULTS.results

    out = np.zeros((B, N, D), np.float64)
    for c in range(NCORES):
        out[c // 4] += res[c]["out"].astype(np.float64)
    return out.astype(np.float32)


# revision 8
# speedup vs baseline: 1.1807x; 1.1511x over previous
"""Multi-head attention TRN2 kernel (8 NeuronCores, SPMD).

Problem: B=2, N=2048, D=1024, H=16 heads of dim 64, fp32, per-(b,h)
key-length masking (valid_len, length 32).

Sharding: batch*heads across 8 cores - core c handles batch b=c//4 and 4
heads ("slots", rank-aligned by valid_len so the SPMD trip counts stay
balanced).  Per core:

  phase P (projections, bf16 inputs to halve HBM traffic):
    K^T/Q^T = Wslice^T @ x^T   (head dims on partitions, positions free)
    V       = x^T-tiles as lhsT, Wv as rhs  (positions on partitions),
              copied into V1 = [V_j | 1] blocks per (key-tile, slot)
  phase A (attention, f32r):
    S^T   = K^T.T @ Q^T per (slot, key-tile), batched in PSUM pairs
    P^T   = exp(S^T/8 + bias) on ScalarE - the valid_len mask is a
            per-partition bias column (0 / -30000), so no V masking ops;
            key-tile pairs that are uniformly valid/invalid across all
            cores share one exp instruction ([128,1024] batch)
    acc   = V1.T @ P^T accumulated over key tiles (ones column gives the
            softmax denominator as row 64)
    normalize via DVE reciprocal + Newton, GpSimd partition broadcast
    out_partial = heads^T.T @ Wo_slice -> (2048, 1024) per core
Host sums the 4 per-core partials of each batch element (the unshard for
the row-sharded Wo) and gathers.

Matmuls: projections run bf16*bf16 (f32 accumulate); attention runs
float32r (full PE rate at free>=256).
"""
import sys
import numpy as np
from contextlib import ExitStack

sys.path.insert(0, "/opt/trn_rl_repo")

import concourse.bass as bass  # noqa: E402
from concourse import bacc, mybir  # noqa: E402
import concourse.tile as tile  # noqa: E402
from concourse.bass_utils import run_bass_kernel_spmd  # noqa: E402

F32 = mybir.dt.float32
F32R = mybir.dt.float32r
BF16 = mybir.dt.bfloat16
AF = mybir.ActivationFunctionType
NPBF16 = mybir.dt.np(BF16)

B, N, D, H = 2, 2048, 1024, 16
DH = 64
HPC = 4          # heads (slots) per core
NCORES = 8
QC = 512         # q chunk (matmul free dim)
NKT = N // 128   # 16 k tiles
NDC = D // 128   # 8 contraction chunks
MASK_BIAS = -30000.0

LAST_RESULTS = None  # BassKernelResults of the most recent run (for tooling)


def _build_program(trips, plans):
    """trips: 4 ints (k-tile count per slot); plans: per slot, list of
    (t0, ntiles) exp-batch groups covering range(trips[j])."""
    nc = bacc.Bacc("TRN2", target_bir_lowering=False, debug=False,
                   num_devices=NCORES)

    xTq = nc.dram_tensor("xTq", [D, N], BF16, kind="ExternalInput")
    xTk = nc.dram_tensor("xTk", [D, N], BF16, kind="ExternalInput")
    xTv = nc.dram_tensor("xTv", [D, N], BF16, kind="ExternalInput")
    wq = nc.dram_tensor("wq", [128, NDC * 256], BF16, kind="ExternalInput")
    wk = nc.dram_tensor("wk", [128, NDC * 256], BF16, kind="ExternalInput")
    wv = nc.dram_tensor("wv", [128, NDC * 256], BF16, kind="ExternalInput")
    wo = nc.dram_tensor("wo", [256, D], F32R, kind="ExternalInput")
    vmask = nc.dram_tensor("vmask", [128, HPC * NKT], F32, kind="ExternalInput")
    out = nc.dram_tensor("out", [N, D], F32, kind="ExternalOutput")

    with tile.TileContext(nc) as tc:
        with ExitStack() as ctx:
            wpool = ctx.enter_context(tc.tile_pool(name="wpool", bufs=1))
            xpool = ctx.enter_context(tc.tile_pool(name="xpool", bufs=6))
            qkpool = ctx.enter_context(tc.tile_pool(name="qkpool", bufs=1))
            v1pool = ctx.enter_context(tc.tile_pool(name="v1pool", bufs=1))
            ptpool = ctx.enter_context(tc.tile_pool(name="ptpool", bufs=4))
            nrmpool = ctx.enter_context(tc.tile_pool(name="nrmpool", bufs=2))
            pbpool = ctx.enter_context(tc.tile_pool(name="pbpool", bufs=1))
            opool = ctx.enter_context(tc.tile_pool(name="opool", bufs=8))

            # only wk is needed before the first matmul; the other weight
            # loads are issued just before their consuming phase so they
            # don't delay the first xk chunks
            t_wk = wpool.tile([128, NDC * 256], BF16, tag="wk")
            nc.sync.dma_start(t_wk[:], wk[:])
            t_wq = wpool.tile([128, NDC * 256], BF16, tag="wq")
            t_wv = wpool.tile([128, NDC * 256], BF16, tag="wv")
            t_wo = [wpool.tile([128, D], F32R, tag=f"wo{p}", name=f"t_wo{p}")
                    for p in range(2)]
            t_vm = wpool.tile([128, HPC * NKT], F32, tag="vm")

            # K^T/Q^T: [128 dims (2 slots), N] per slot-pair
            t_kT = [qkpool.tile([128, N], F32R, tag=f"kT{p}", name=f"t_kT{p}")
                    for p in range(2)]
            t_qT = [qkpool.tile([128, N], F32R, tag=f"qT{p}", name=f"t_qT{p}")
                    for p in range(2)]
            # V1: per key-tile t, 4 blocks of [V_j (64 cols) | ones (1 col)]
            t_v1 = v1pool.tile([128, NKT * HPC * 65], F32R, tag="v1")
            # normalized heads^T per slot pair: [128 dims, N]
            t_pb = [pbpool.tile([128, N], F32R, tag=f"pb{p}", name=f"t_pb{p}")
                    for p in range(2)]

            # ones columns of V1, one strided memset (f32 view: f32r is not
            # a valid memset value type)
            ones_ap = t_v1[:].bitcast(F32).rearrange(
                "p (b c) -> p b c", c=65)[:, :, 64:65]
            nc.vector.memset(ones_ap, 1.0)

            # ---- phase P: projections (K, Q, V) ----
            with tc.tile_pool(name="pp", bufs=8, space="PSUM") as pp:
                for si, (xin, wsb, dsts) in enumerate(
                        ((xTk, t_wk, t_kT), (xTq, t_wq, t_qT))):
                    accs = [pp.tile([128, QC], F32, tag="acc", name=f"acc_{i}")
                            for i in range(8)]
                    for c in range(NDC):
                        xt = xpool.tile([128, N], BF16, tag="xt")
                        nc.sync.dma_start(xt[:], xin[c * 128:(c + 1) * 128, :])
                        for m in range(2):
                            for qq in range(4):
                                nc.tensor.matmul(
                                    accs[m * 4 + qq][:],
                                    wsb[:, c * 256 + m * 128:
                                        c * 256 + (m + 1) * 128],
                                    xt[:, qq * QC:(qq + 1) * QC],
                                    start=(c == 0), stop=(c == NDC - 1))
                    if si == 0:
                        nc.sync.dma_start(t_wq[:], wq[:])
                    else:
                        nc.sync.dma_start(t_wv[:], wv[:])
                    for i in range(8):
                        dst = dsts[i // 4][:, (i % 4) * QC:(i % 4 + 1) * QC]
                        with nc.allow_low_precision(reason="f32r 4B"):
                            nc.any.tensor_copy(dst, accs[i][:])
                nc.sync.dma_start(t_wo[0][:], wo[0:128, :])
                nc.sync.dma_start(t_wo[1][:], wo[128:256, :])
                nc.sync.dma_start(t_vm[:], vmask[:])
                # V projection: two half-column passes of 8 k-tiles
                for g in range(2):
                    accs = [pp.tile([128, 256], F32, tag="acc",
                                    name=f"accv_{i}") for i in range(8)]
                    for c in range(NDC):
                        xt = xpool.tile([128, 1024], BF16, tag="xtv")
                        nc.sync.dma_start(
                            xt[:], xTv[c * 128:(c + 1) * 128,
                                       g * 1024:(g + 1) * 1024])
                        for kt8 in range(8):
                            nc.tensor.matmul(
                                accs[kt8][:],
                                xt[:, kt8 * 128:(kt8 + 1) * 128],
                                t_wv[:, c * 256:(c + 1) * 256],
                                start=(c == 0), stop=(c == NDC - 1))
                    for kt8 in range(8):
                        t = g * 8 + kt8
                        # [128, 4, 64] strided copy: slot j -> V1 block
                        src = accs[kt8][:].rearrange("p (j c) -> p j c", c=64)
                        dst = t_v1[:, t * 260:(t + 1) * 260].rearrange(
                            "p (j c) -> p j c", c=65)[:, :, 0:64]
                        with nc.allow_low_precision(reason="f32r 4B"):
                            nc.vector.tensor_copy(dst, src)

            # ---- phase A: attention with fused output projection ----
            with tc.tile_pool(name="ap", bufs=1, space="PSUM") as ap:
                for q in range(N // QC):
                    qs = slice(q * QC, (q + 1) * QC)
                    for j in range(HPC):
                        p, half = j // 2, j % 2
                        rows = slice(half * 64, (half + 1) * 64)
                        acc = ap.tile([65, QC], F32, tag="acc2", bufs=4,
                                      name=f"acc_{j}")
                        plan = plans[j]
                        for gi, (t0, nt) in enumerate(plan):
                            sT = ap.tile([128, nt * QC], F32, tag="sT", bufs=2)
                            for i in range(nt):
                                t = t0 + i
                                nc.tensor.matmul(
                                    sT[:, i * QC:(i + 1) * QC],
                                    t_kT[p][rows, t * 128:(t + 1) * 128],
                                    t_qT[p][rows, qs],
                                    start=True, stop=True)
                            pT = ptpool.tile([128, nt * QC], F32R, tag="pT")
                            nc.scalar.activation(
                                pT[:], sT[:], AF.Exp, scale=0.125,
                                bias=t_vm[:, j * NKT + t0: j * NKT + t0 + 1])
                            for i in range(nt):
                                t = t0 + i
                                base = (t * HPC + j) * 65
                                nc.tensor.matmul(
                                    acc[:], t_v1[:, base: base + 65],
                                    pT[:, i * QC:(i + 1) * QC],
                                    start=(gi == 0 and i == 0),
                                    stop=(gi == len(plan) - 1 and i == nt - 1))
                        # normalize (recip + one Newton step, bcast on GpSimd)
                        r0 = nrmpool.tile([1, QC], F32, tag="r0")
                        nc.vector.reciprocal(r0[:], acc[64:65, :])
                        t1 = nrmpool.tile([1, QC], F32, tag="t1")
                        nc.vector.tensor_mul(t1[:], acc[64:65, :], r0[:])
                        t2 = nrmpool.tile([1, QC], F32, tag="t2")
                        nc.vector.tensor_scalar(
                            t2[:], t1[:], -1.0, 2.0,
                            mybir.AluOpType.mult, mybir.AluOpType.add)
                        r1 = nrmpool.tile([1, QC], F32, tag="r1")
                        nc.vector.tensor_mul(r1[:], r0[:], t2[:])
                        bc_sb = nrmpool.tile([64, QC], F32, tag="bc_sb")
                        nc.gpsimd.partition_broadcast(bc_sb[:], r1[:])
                        with nc.allow_low_precision(reason="f32r 4B"):
                            nc.vector.tensor_mul(
                                t_pb[p][rows, qs], acc[0:64, :], bc_sb[:])
                    # output projection for the 4 q-tiles of this chunk
                    for qt in range(q * (QC // 128), (q + 1) * (QC // 128)):
                        ts = slice(qt * 128, (qt + 1) * 128)
                        stage = opool.tile([128, D], F32, tag="ostage")
                        for ch in range(2):
                            o_ps = ap.tile([128, 512], F32, tag="sT", bufs=2)
                            for p2 in range(2):
                                nc.tensor.matmul(
                                    o_ps[:], t_pb[p2][:, ts],
                                    t_wo[p2][:, ch * 512:(ch + 1) * 512],
                                    start=(p2 == 0), stop=(p2 == 1))
                            nc.any.tensor_copy(
                                stage[:, ch * 512:(ch + 1) * 512], o_ps[:])
                        nc.sync.dma_start(out[ts, :], stage[:])

    nc.finalize()
    return nc


def _make_plans(trips, vls_by_slot):
    """Greedy pair batching: (t, t+1) share one exp iff every core's vl is
    outside the open interval (128*t, 128*(t+2)) - then one bias column
    describes both tiles on every core."""
    plans = []
    for j in range(HPC):
        plan, t = [], 0
        while t < trips[j]:
            if t + 1 < trips[j] and all(
                    v <= 128 * t or v >= 128 * (t + 2)
                    for v in vls_by_slot[j]):
                plan.append((t, 2))
                t += 2
            else:
                plan.append((t, 1))
                t += 1
        plans.append(plan)
    return plans


def kernel(queries, keys, values, valid_len, Wq, Wk, Wv, Wo):
    global LAST_RESULTS
    queries = np.asarray(queries, dtype=np.float32)
    keys = np.asarray(keys, dtype=np.float32)
    values = np.asarray(values, dtype=np.float32)
    Wq = np.asarray(Wq, dtype=np.float32)
    Wk = np.asarray(Wk, dtype=np.float32)
    Wv = np.asarray(Wv, dtype=np.float32)
    Wo = np.asarray(Wo, dtype=np.float32)
    vl = np.asarray(valid_len).astype(np.int64).reshape(B * H)

    # rank-aligned slot assignment: per batch, heads sorted by vl desc;
    # slot j of the 4 cores of that batch takes ranks 4j..4j+3
    order = {}
    for b in range(B):
        idx = (np.argsort(-vl[b * H:(b + 1) * H], kind="stable") + b * H)
        for cg in range(4):
            order[b * 4 + cg] = [int(idx[4 * j + cg]) for j in range(HPC)]
    trips, vls_by_slot = [], []
    for j in range(HPC):
        vs = [int(vl[order[c][j]]) for c in range(NCORES)]
        vls_by_slot.append(vs)
        m = max(-(-v // 128) for v in vs)
        trips.append(max(1, min(NKT, m)))
    plans = _make_plans(trips, vls_by_slot)

    nc = _build_program(tuple(trips), plans)

    in_maps = []
    for c in range(NCORES):
        b = c // 4
        heads = order[c]
        cols = np.concatenate(
            [np.arange((h - b * H) * DH, (h - b * H + 1) * DH) for h in heads])

        def wlayout(w):
            return np.ascontiguousarray(
                w[:, cols].reshape(NDC, 128, 256).transpose(1, 0, 2)
                .reshape(128, NDC * 256).astype(NPBF16))

        vm = np.zeros((128, HPC * NKT), np.float32)
        for j, h in enumerate(heads):
            bias = np.where(np.arange(N) < vl[h], 0.0, MASK_BIAS)
            vm[:, j * NKT:(j + 1) * NKT] = bias.reshape(NKT, 128).T

        in_maps.append({
            "xTq": np.ascontiguousarray(queries[b].T.astype(NPBF16)),
            "xTk": np.ascontiguousarray(keys[b].T.astype(NPBF16)),
            "xTv": np.ascontiguousarray(values[b].T.astype(NPBF16)),
            "wq": wlayout(Wq),
            "wk": wlayout(Wk),
            "wv": wlayout(Wv),
            "wo": np.ascontiguousarray(Wo[cols, :]),
            "vmask": vm,
        })

    LAST_RESULTS = run_bass_kernel_spmd(nc, in_maps, list(range(NCORES)))
    res = LAST_RESULTS.results

    out = np.zeros((B, N, D), np.float64)
    for c in range(NCORES):
        out[c // 4] += res[c]["out"].astype(np.float64)
    return out.astype(np.float32)
